# revision 38
# baseline (speedup 1.0000x reference)
"""
MinibatchDiscrimination kernel for 8x TRN2 NeuronCores (Bass/Tile).

Math:  x = inputs @ T  -> [B, K, D] with B=512, K=100, D=5
       out[i,k] = sum_j exp(-sum_d |x[i,k,d]-x[j,k,d]|)

Strategy — symmetric block-tournament over the pairwise matrix:

  The B x B pairwise matrix is tiled into 8x8 blocks of 64x64 (one row-group
  per core). Each unordered block-pair only needs computing once: from one
  computed block, ROW sums come from the ACT accumulator (or a DVE reduce)
  and COLUMN sums (= row sums of the transposed block, by symmetry of the
  L1 distance) come from a Pool-engine (GPSIMD) accumulation over the exp
  tiles. Core c computes blocks (c, c+k) for k=0..4 (mod 8) with ragged
  per-row column ranges [j, E(j)):

    - local col order: [diag 0..64 | k=1,2,3 at 64..256 | dist-4 high half
      (c+4 rows 32..64) at 256..288 | dist-4 low half DESCENDING (288+s
      holds c+4's row 31-s) at 288..320]
    - diag block: upper triangle only (cols >= j); the lower triangle is
      recovered from the diag columns of the colacc by symmetry (minus the
      double-counted self term exp(0)=1, subtracted on the host)
    - k=1,2,3: row sums kept locally + column sums exchanged to core c+k
      via the host during output assembly
    - dist-4 block: rows j<32 compute cols 256..320-j (high half + the low
      half strictly above the antidiagonal); rows j>=32 compute 256..288
      only. The remaining low-half pairs come from core c-4's colacc over
      its rows j' < i (the colacc add range ends at 319-j, strict), and
      rows 32..64 get the high-half transpose from core c-4's colacc over
      its rows <32.

  Row j of core q then receives: own row sums plus exchanged column sums
  from cores q-1, q-2, q-3, q-4 — every pair exactly once.

Per core c of 8 (rolled by 64c so the program is SPMD-identical):
  - xT[kd, i] = sum_f T[f, kd] * inT[f, i] on PE (4 chunks of 125 kd).
  - Per output row j (cols [j, E(j))):
      ab_c[p, i] = relu(xT_c[p, i] - xT_c[p, j])  (DVE tensor_scalar
                   (subtract, max 0.0), fp16 4x perf mode; the per-partition
                   scalar is an f32 upcast of the fp16 xT column so the
                   diagonal is exactly 0)
      dist[32c+m, :] = 2*sum_d ab[5m+d, :]        (PE d-sum matmul with a
                   0/2 block matrix, col-tiled per chunk, start=True)
      dist += -S[k, i]                            (PE negI matmul, emitted
                   last so early rows never stall on the S16 chain)
      dump[:, :]  = exp(-dist - S_j) fp16 -> SBUF (ACT; bias = -S_j per
                   partition; accum_out row sums except the last NRED rows,
                   which use DVE tensor_reduce over the fp16 dump instead —
                   saves the fixed 187ns ACT accumulator-read where ACT is
                   the tighter engine)
      colacc[:, j:CP] += dump[:, j:CP]            (Pool tensor_tensor add,
                   f32 accumulator in SBUF — column sums entirely off
                   PE/ACT/DVE)
  - dist row p=32c+m holds k=25c+m (m<25); host transposes/reassembles and
    adds the exchanged column-sum blocks.

  Hardware notes (CoreSim cost model, HW-validated structure):
  - Per-instruction costs: DVE tensor_scalar fp16 = 0.26*W + 60ns (4x
    mode); PE matmul fp16 = 0.4167*W; ACT exp = 0.833*W + 185 (+187 with
    accum_out); Pool ops ~0.9ns/col + 95ns Q7 launch; DVE tensor_reduce
    runs at 1x (1.04*W + 60).
  - ab/dump tiles are STATIC rings (no cross-iteration WAW deps): DVE
    instructions carry no waits in steady state.
  - The PE p-state ramp clock is reset by any multi-wait PE instruction
    (bacc splits it into an EventSemaphore): 6 single-wait "gate" matmuls
    absorb each input-DMA semaphore so every real matmul carries at most
    one wait and the whole input stage runs at full clock.
  - Input DMAs are spread across the SP/ACT/Pool queues (a DMA's transfer
    time is charged to its issuing queue) so all inputs land by ~3.5us.
"""

import contextlib
import sys
import numpy as np

for _p in ("/opt/trn_rl_repo",):
    if _p not in sys.path:
        sys.path.insert(0, _p)

B = 512
F = 1024
K = 100
D = 5
KD = K * D  # 500
NCORES = 8
JPC = B // NCORES  # 64 output rows per core
NCHUNK = 4  # kd chunks of 125
CHUNK = KD // NCHUNK  # 125
KPC = K // NCHUNK  # 25 k's per chunk
NBLK = 5  # col block-groups computed per core (k = 0..4)
W = NBLK * JPC  # 320 pairwise columns per core
NEX = 3  # exchanged colsum groups (k = 1, 2, 3)
CEX = NEX * JPC  # 192 exchanged columns (local cols 64..256)
D4HI = 4 * JPC  # 256: start of the distance-4 high-half cols
D4LO = D4HI + 32  # 288: start of the distance-4 low-half cols (descending)
NCOL = W  # 320 columns exported in colout
NRED = 20  # rows 64-NRED..63 get DVE tensor_reduce row sums (no ACT accum)

_NC_CACHE = {}


def _erow(j):
    # row j computes cols [j, 320-j): the whole distance-4 block (cols
    # 256..320) is ordered DESCENDING by partner row (col 256+s holds core
    # c+4's row 63-s), so the range covers exactly the partners t >= j;
    # pairs with t < j come from core c-4's colacc (strict antidiagonal)
    return W - j


def build_nc():
    import concourse.bass as bass
    import concourse.bacc as bacc
    import concourse.mybir as mybir
    from concourse.tile import TileContext

    nc = bacc.Bacc(None, target_bir_lowering=False, debug=True)

    inT = nc.declare_dram_parameter("inT", [F, W], mybir.dt.float16, isOutput=False)
    Tm = nc.declare_dram_parameter("Tm", [F, KD], mybir.dt.float16, isOutput=False)
    # dmat[5m+d, m] = 2.0 (d-sum of 2*relu), dmat[5m+d, 32+m] = 1.0 (S row sums)
    dmat = nc.declare_dram_parameter(
        "dmat", [CHUNK, 64], mybir.dt.float16, isOutput=False
    )
    negI = nc.declare_dram_parameter("negI", [128, 128], mybir.dt.float16, isOutput=False)
    rowsum = nc.declare_dram_parameter("rowsum", [128, JPC], mybir.dt.float32, isOutput=True)
    colout = nc.declare_dram_parameter(
        "colout", [128, NCOL], mybir.dt.float32, isOutput=True
    )

    with TileContext(nc) as tc:
        with tc.tile_pool(name="persist", bufs=1) as pp:
            T_sb = pp.tile([128, 8 * KD], mybir.dt.float16, name="T_sb")
            inT_sb = pp.tile([128, 8 * W], mybir.dt.float16, name="inT_sb")
            dmat_sb = pp.tile([CHUNK, 64], mybir.dt.float16, name="dmat_sb")
            negI_sb = pp.tile([128, 128], mybir.dt.float16, name="negI_sb")
            S16_sb = pp.tile([128, W], mybir.dt.float16, name="S16_sb")
            negSj_sb = pp.tile([128, JPC], mybir.dt.float32, name="negSj_sb")
            colacc_sb = pp.tile([128, NCOL], mybir.dt.float32, name="colacc_sb")
            xT_sb = pp.tile([128, NCHUNK * W], mybir.dt.float16, name="xT_sb")
            # f32 upcasts of xT columns 0..JPC (tensor_scalar per-partition
            # scalars must be f32), from the fp16 xT so the diagonal is 0
            xTj_sb = pp.tile([128, NCHUNK * JPC], mybir.dt.float32, name="xTj_sb")
            raw_sb = pp.tile([128, JPC], mybir.dt.float32, name="raw_sb")
            warm_sb = pp.tile([1, 1], mybir.dt.float32, name="warm_sb")

            # Static rings: every (j, chunk) gets its own ab tile; the dump
            # ring must cover the tail rows reduced on DVE after the loop.
            ab_ring = [
                pp.tile(
                    [CHUNK, _erow(t // NCHUNK) - (t // NCHUNK)],
                    mybir.dt.float16,
                    name=f"ab{t}",
                )
                for t in range(JPC * NCHUNK)
            ]
            NDUMP = 24
            dump_ring = [
                pp.tile([128, W], mybir.dt.float16, name=f"dump{t}")
                for t in range(NDUMP)
            ]

            # --- input DMAs, spread across the three DMA-capable queues ---
            def dma_T(e, h):
                e.dma_start(
                    out=T_sb[:, h * 4 * KD : (h + 1) * 4 * KD].rearrange(
                        "p (t k) -> p t k", t=4
                    ),
                    in_=Tm[h * 512 : (h + 1) * 512, :].rearrange(
                        "(t p) k -> p t k", t=4
                    ),
                )

            def dma_inT(e, h):
                e.dma_start(
                    out=inT_sb[:, h * 4 * W : (h + 1) * 4 * W].rearrange(
                        "p (t w) -> p t w", t=4
                    ),
                    in_=inT[h * 512 : (h + 1) * 512, :].rearrange(
                        "(t p) w -> p t w", t=4
                    ),
                )

            dma_T(nc.sync, 0)
            dma_inT(nc.gpsimd, 0)
            dma_T(nc.gpsimd, 1)
            dma_inT(nc.sync, 1)
            nc.sync.dma_start(out=dmat_sb[:, :], in_=dmat[:, :])
            nc.sync.dma_start(out=negI_sb[:, :], in_=negI[:, :])
            # zero the Pool-side column-sum accumulator while DMAs run
            nc.gpsimd.memset(colacc_sb[:, :], 0.0)
            # warm the ACT exp table while the xT matmuls run (~1.3us)
            nc.vector.memset(warm_sb[:, :], 0.0)
            nc.scalar.activation(
                warm_sb[:, :], warm_sb[:, :], mybir.ActivationFunctionType.Exp
            )

            with tc.tile_pool(name="xtps", bufs=3, space="PSUM") as xtps:
                # single-wait gates: absorb each input-DMA semaphore so no
                # real matmul carries two waits (a 2-wait matmul gets an
                # EventSemaphore that resets the PE p-state ramp, halving
                # the input-stage matmul clock)
                gate_srcs = [
                    T_sb[:, 0:64],
                    inT_sb[:, 0:64],
                    T_sb[:, 4 * KD : 4 * KD + 64],
                    inT_sb[:, 4 * W : 4 * W + 64],
                    dmat_sb[:, 0:64],
                    negI_sb[:, 0:64],
                ]
                g_ps0 = xtps.tile([128, 64], mybir.dt.float32, name="gate_a", bufs=1)
                g_ps1 = xtps.tile([128, 64], mybir.dt.float32, name="gate_b", bufs=1)
                for gi, gsrc in enumerate(gate_srcs):
                    g_ps = g_ps0 if gi < 4 else g_ps1
                    q = gi % 4
                    nc.tensor.matmul(
                        g_ps[32 * q : 32 * q + 32, :],
                        gsrc[:, 0:32],
                        gsrc[:, 0:64],
                        start=True,
                        stop=True,
                        tile_position=(0, 32 * q),
                        skip_group_check=True,
                    )

                # --- xT chunks: xT[kd, i] via PE over f tiles; each chunk's
                # S-row-sum quadrant matmul is emitted right after its copy
                # so the S16/negSj chain never serializes at the end ---
                S_ps = xtps.tile([128, W], mybir.dt.float32, name="S_ps", bufs=1)
                for c in range(NCHUNK):
                    xt_ps = xtps.tile([CHUNK, W], mybir.dt.float32, name="xt_ps")
                    for t in range(8):
                        nc.tensor.matmul(
                            xt_ps[:, :],
                            T_sb[:, t * KD + c * CHUNK : t * KD + (c + 1) * CHUNK],
                            inT_sb[:, t * W : (t + 1) * W],
                            start=(t == 0),
                            stop=(t == 7),
                        )
                    # all fp16 copies on DVE: ACT is the binding engine and
                    # DVE's late-stream reduce stalls absorb the extra work
                    nc.vector.tensor_copy(
                        xT_sb[0:CHUNK, c * W : (c + 1) * W], xt_ps[:, :]
                    )
                    nc.vector.tensor_copy(
                        xTj_sb[0:CHUNK, c * JPC : (c + 1) * JPC],
                        xT_sb[0:CHUNK, c * W : c * W + JPC],
                    )
                    # S[k, i] = sum_d x[i,k,d] at partitions 32c+m
                    nc.tensor.matmul(
                        S_ps[32 * c : 32 * c + 32, :],
                        dmat_sb[:, 32:64],
                        xT_sb[0:CHUNK, c * W : (c + 1) * W],
                        start=True,
                        stop=True,
                        tile_position=(0, 32 * c),
                        skip_group_check=True,
                    )
                    # per-quadrant S16 copy so the fp16 S is complete right
                    # after the last chunk's S matmul; split across ACT and
                    # DVE to take 0.9us off the binding ACT queue
                    if c % 2 == 0:
                        nc.scalar.copy(
                            S16_sb[32 * c : 32 * c + 32, :],
                            S_ps[32 * c : 32 * c + 32, :],
                        )
                    else:
                        nc.vector.tensor_copy(
                            S16_sb[32 * c : 32 * c + 32, :],
                            S_ps[32 * c : 32 * c + 32, :],
                        )
                # exp bias column: -S_j, upcast from the SAME fp16 S16 the
                # negI matmul reads so the diagonal cancels exactly; on ACT
                # so the DVE queue is never stalled ahead of the first ts
                nc.scalar.mul(negSj_sb[:, :], S16_sb[:, 0:JPC], -1.0)

            mainps_es = contextlib.ExitStack()
            mainps = mainps_es.enter_context(
                tc.tile_pool(name="mainps", bufs=1, space="PSUM")
            )
            NDIST = 6
            dist_bufs = [
                mainps.tile([128, W], mybir.dt.float32, name=f"dist{i}")
                for i in range(NDIST)
            ]

            def emit_ts(j, c, eng):
                E = _erow(j)
                eng.tensor_scalar(
                    ab_ring[j * NCHUNK + c][:, :],
                    xT_sb[0:CHUNK, c * W + j : c * W + E],
                    xTj_sb[0:CHUNK, c * JPC + j : c * JPC + j + 1],
                    0.0,
                    mybir.AluOpType.subtract,
                    mybir.AluOpType.max,
                )

            # chunk-3 relu for EVEN rows runs on the otherwise-idle Pool
            # engine (~440ns there vs ~125ns of DVE time saved; the total
            # is DVE-work-bound). Emitted 4 rows ahead so the PE d-sum
            # never waits on the slower Pool.
            emit_ts(0, NCHUNK - 1, nc.gpsimd)
            emit_ts(2, NCHUNK - 1, nc.gpsimd)

            # --- main loop over output rows ---
            for j in range(JPC):
                E = _erow(j)
                # colacc: diag (lower-tri by symmetry), k123, and the
                # distance-4 cols STRICTLY above the antidiagonal (the col
                # range ends one short of E so the t=j partner, already in
                # the own row sum, is not double-counted via the exchange)
                CP = W - 1 - j
                dist = dist_bufs[j % NDIST]
                if j + 4 < JPC and (j + 4) % 2 == 0:
                    emit_ts(j + 4, NCHUNK - 1, nc.gpsimd)
                for c in range(NCHUNK):
                    # ab = relu(xT[:, i] - xT[:, j]) : (in - s1) max 0.0
                    # (const scalar2 keeps the second DVE read port free so
                    # the 4x perf mode applies)
                    if not (c == NCHUNK - 1 and j % 2 == 0):
                        emit_ts(j, c, nc.vector)
                    # dist[32c+m, :] = 2 * sum_d ab[5m+d, :]
                    nc.tensor.matmul(
                        dist[32 * c : 32 * c + 32, j:E],
                        dmat_sb[:, 0:32],
                        ab_ring[j * NCHUNK + c][:, :],
                        start=True,
                        stop=False,
                        tile_position=(0, 32 * c),
                        skip_group_check=True,
                    )
                # dist += -S[k, i], LAST so early rows don't stall on the
                # S16 chain during the input stage
                nc.tensor.matmul(
                    dist[:, j:E],
                    negI_sb[:, :],
                    S16_sb[:, j:E],
                    start=False,
                    stop=True,
                    skip_group_check=True,
                )
                # dump = exp(-dist - S_j) = exp(-L1(i,j)) fp16; ACT accum
                # gives the row sums except for the last NRED rows (DVE
                # tensor_reduces after the loop save the fixed 187ns ACT
                # accumulator-read there)
                nc.scalar.activation(
                    dump_ring[j % NDUMP][:, j:E],
                    dist[:, j:E],
                    mybir.ActivationFunctionType.Exp,
                    bias=negSj_sb[:, j : j + 1],
                    scale=-1.0,
                    accum_out=(raw_sb[:, j : j + 1] if j < JPC - NRED else None),
                )
                if j == 33:
                    # these outputs are final: row sums 0..32 (ACT accum),
                    # colacc diag cols 0..32 (adds stop at j=i) and cols
                    # 288..320 (dist-4 cols for partners <32, adds stop at
                    # j=31) — ship them now to shrink the end-of-loop DMA
                    nc.sync.dma_start(out=rowsum[:, 0:32], in_=raw_sb[:, 0:32])
                    nc.sync.dma_start(out=colout[:, 0:32], in_=colacc_sb[:, 0:32])
                    nc.sync.dma_start(out=colout[:, D4LO:W], in_=colacc_sb[:, D4LO:W])
                # column sums on the otherwise-idle Pool (GPSIMD) engine:
                # colacc += dump[:, j:CP] (f32 accumulator in SBUF)
                nc.gpsimd.tensor_tensor(
                    colacc_sb[:, j:CP],
                    colacc_sb[:, j:CP],
                    dump_ring[j % NDUMP][:, j:CP],
                    mybir.AluOpType.add,
                )
            for j in range(JPC - NRED, JPC):
                nc.vector.tensor_reduce(
                    raw_sb[:, j : j + 1],
                    dump_ring[j % NDUMP][:, j : _erow(j)],
                    mybir.AxisListType.X,
                    mybir.AluOpType.add,
                )
            mainps_es.close()
            nc.sync.dma_start(out=rowsum[:, 32:JPC], in_=raw_sb[:, 32:JPC])
            nc.gpsimd.dma_start(out=colout[:, 32:D4LO], in_=colacc_sb[:, 32:D4LO])

    nc.finalize()
    return nc


def _aux_consts():
    dm = np.zeros([CHUNK, 64], dtype=np.float16)
    for m in range(KPC):
        dm[5 * m : 5 * m + 5, m] = 2.0
        dm[5 * m : 5 * m + 5, 32 + m] = 1.0
    negI = (-np.eye(128)).astype(np.float16)
    return dm, negI


def make_in_maps(inputs, T):
    f16 = np.float16
    Tm = np.asarray(T, dtype=np.float32).astype(f16)
    dm, negI = _aux_consts()
    in_maps = []
    x = np.asarray(inputs, dtype=np.float32)
    for c in range(NCORES):
        rolled = np.roll(x, -JPC * c, axis=0)[0:W, :]
        # local col order: [diag+k123 (0..256) | c+4 rows 63..0 descending
        # (256..320): col 256+s holds c+4's row 63-s]
        rolled = np.concatenate([rolled[0:D4HI], rolled[D4HI:W][::-1]], axis=0)
        inTc = np.ascontiguousarray(rolled.T).astype(f16)
        in_maps.append(
            {
                "inT": inTc,
                "Tm": Tm,
                "dmat": dm,
                "negI": negI,
            }
        )
    return in_maps


def assemble_output(results):
    out = np.zeros([B, K], dtype=np.float32)
    # own row sums: raw[32c+m, j] -> out[64q+j, 25c+m]
    for q in range(NCORES):
        raw = np.asarray(results[q]["rowsum"], dtype=np.float32)  # [128, JPC]
        for cc in range(NCHUNK):
            out[JPC * q : JPC * (q + 1), KPC * cc : KPC * (cc + 1)] = raw[
                32 * cc : 32 * cc + KPC, :
            ].T
    # column sums: core b's group k serves rows of core b+k. k=0 is the own
    # diag block (lower triangle by symmetry; subtract the double-counted
    # self term exp(0)=1), k=1..3 are the exchanged off-diag groups.
    for b in range(NCORES):
        col = np.asarray(results[b]["colout"], dtype=np.float32)  # [128, NCOL]
        for k in range(0, NEX + 1):
            q = (b + k) % NCORES
            blk = col[:, JPC * k : JPC * (k + 1)]  # [128, JPC]
            for cc in range(NCHUNK):
                out[JPC * q : JPC * (q + 1), KPC * cc : KPC * (cc + 1)] += blk[
                    32 * cc : 32 * cc + KPC, :
                ].T
        # distance-4 block (descending): col 256+s serves row 63-s of core
        # b+4 with colsums over core b's rows j < 63-s (strict antidiagonal)
        q = (b + 4) % NCORES
        blk = col[:, D4HI:W][:, ::-1]  # reversed: col t serves row t of b+4
        for cc in range(NCHUNK):
            out[JPC * q : JPC * (q + 1), KPC * cc : KPC * (cc + 1)] += blk[
                32 * cc : 32 * cc + KPC, :
            ].T
    out -= 1.0
    return out


def kernel(inputs, T):
    from concourse.bass_utils import run_bass_kernel_spmd

    if "nc" not in _NC_CACHE:
        _NC_CACHE["nc"] = build_nc()
    nc = _NC_CACHE["nc"]
    in_maps = make_in_maps(inputs, T)
    res = run_bass_kernel_spmd(nc, in_maps, list(range(NCORES)))
    return assemble_output(res.results)


if __name__ == "__main__":
    sys.path.insert(0, "/root/problem")
    from reference import setup_inputs, reference

    inputs = setup_inputs()
    expected = np.asarray(reference(**inputs))
    actual = kernel(**{k: np.asarray(v) for k, v in inputs.items()})
    err = np.abs(actual - expected)
    rel = np.linalg.norm(actual - expected) / np.linalg.norm(expected)
    print(f"max abs err: {err.max():.3e}")
    print(f"Relative error: {rel:.3e}")


# revision 40
# speedup vs baseline: 1.0011x; 1.0011x over previous
"""
MinibatchDiscrimination kernel for 8x TRN2 NeuronCores (Bass/Tile).

Math:  x = inputs @ T  -> [B, K, D] with B=512, K=100, D=5
       out[i,k] = sum_j exp(-sum_d |x[i,k,d]-x[j,k,d]|)

Strategy — symmetric block-tournament over the pairwise matrix:

  The B x B pairwise matrix is tiled into 8x8 blocks of 64x64 (one row-group
  per core). Each unordered block-pair only needs computing once: from one
  computed block, ROW sums come from the ACT accumulator (or a DVE reduce)
  and COLUMN sums (= row sums of the transposed block, by symmetry of the
  L1 distance) come from a Pool-engine (GPSIMD) accumulation over the exp
  tiles. Core c computes blocks (c, c+k) for k=0..4 (mod 8) with ragged
  per-row column ranges [j, E(j)):

    - local col order: [diag 0..64 | k=1,2,3 at 64..256 | dist-4 high half
      (c+4 rows 32..64) at 256..288 | dist-4 low half DESCENDING (288+s
      holds c+4's row 31-s) at 288..320]
    - diag block: upper triangle only (cols >= j); the lower triangle is
      recovered from the diag columns of the colacc by symmetry (minus the
      double-counted self term exp(0)=1, subtracted on the host)
    - k=1,2,3: row sums kept locally + column sums exchanged to core c+k
      via the host during output assembly
    - dist-4 block (cols 256..320, DESCENDING: col 256+s holds c+4's row
      63-s): row j covers exactly the partners t >= j via the uniform range
      [j, 320-j); partners t < j come from core c-4's colacc over its rows
      j' < t (the colacc add range ends at 319-j, strict, so the t=j
      partner pair is counted once per side).

  Row j of core q then receives: own row sums plus exchanged column sums
  from cores q-1, q-2, q-3, q-4 — every pair exactly once.

Per core c of 8 (rolled by 64c so the program is SPMD-identical):
  - xT[kd, i] = sum_f T[f, kd] * inT[f, i] on PE (4 chunks of 125 kd).
  - Per output row j (cols [j, E(j))):
      ab_c[p, i] = relu(xT_c[p, i] - xT_c[p, j])  (DVE tensor_scalar
                   (subtract, max 0.0), fp16 4x perf mode; the per-partition
                   scalar is an f32 upcast of the fp16 xT column so the
                   diagonal is exactly 0)
      dist[32c+m, :] = 2*sum_d ab[5m+d, :]        (PE d-sum matmul with a
                   0/2 block matrix, col-tiled per chunk, start=True)
      dist += -S[k, i]                            (PE negI matmul, emitted
                   last so early rows never stall on the S16 chain)
      dump[:, :]  = exp(-dist - S_j) fp16 -> SBUF (ACT; bias = -S_j per
                   partition; accum_out row sums except the last NRED rows,
                   which use DVE tensor_reduce over the fp16 dump instead —
                   saves the fixed 187ns ACT accumulator-read where ACT is
                   the tighter engine)
      colacc[:, j:CP] += dump[:, j:CP]            (Pool tensor_tensor add,
                   f32 accumulator in SBUF — column sums entirely off
                   PE/ACT/DVE)
  - dist row p=32c+m holds k=25c+m (m<25); host transposes/reassembles and
    adds the exchanged column-sum blocks.

  Hardware notes (CoreSim cost model, HW-validated structure):
  - Per-instruction costs: DVE tensor_scalar fp16 = 0.26*W + 60ns (4x
    mode); PE matmul fp16 = 0.4167*W; ACT exp = 0.833*W + 185 (+187 with
    accum_out); Pool ops ~0.9ns/col + 95ns Q7 launch; DVE tensor_reduce
    runs at 1x (1.04*W + 60).
  - ab/dump tiles are STATIC rings (no cross-iteration WAW deps): DVE
    instructions carry no waits in steady state.
  - The PE p-state ramp clock is reset by any multi-wait PE instruction
    (bacc splits it into an EventSemaphore): 6 single-wait "gate" matmuls
    absorb each input-DMA semaphore so every real matmul carries at most
    one wait and the whole input stage runs at full clock.
  - Input DMAs are spread across the SP/ACT/Pool queues (a DMA's transfer
    time is charged to its issuing queue) so all inputs land by ~3.5us.
"""

import contextlib
import sys
import numpy as np

for _p in ("/opt/trn_rl_repo",):
    if _p not in sys.path:
        sys.path.insert(0, _p)

B = 512
F = 1024
K = 100
D = 5
KD = K * D  # 500
NCORES = 8
JPC = B // NCORES  # 64 output rows per core
NCHUNK = 4  # kd chunks of 125
CHUNK = KD // NCHUNK  # 125
KPC = K // NCHUNK  # 25 k's per chunk
NBLK = 5  # col block-groups computed per core (k = 0..4)
W = NBLK * JPC  # 320 pairwise columns per core
NEX = 3  # exchanged colsum groups (k = 1, 2, 3)
CEX = NEX * JPC  # 192 exchanged columns (local cols 64..256)
D4HI = 4 * JPC  # 256: start of the distance-4 high-half cols
D4LO = D4HI + 32  # 288: start of the distance-4 low-half cols (descending)
NCOL = W  # 320 columns exported in colout
NRED = 20  # rows 64-NRED..63 get DVE tensor_reduce row sums (no ACT accum)

_NC_CACHE = {}


def _erow(j):
    # row j computes cols [j, 320-j): the whole distance-4 block (cols
    # 256..320) is ordered DESCENDING by partner row (col 256+s holds core
    # c+4's row 63-s), so the range covers exactly the partners t >= j;
    # pairs with t < j come from core c-4's colacc (strict antidiagonal)
    return W - j


def build_nc():
    import concourse.bass as bass
    import concourse.bacc as bacc
    import concourse.mybir as mybir
    from concourse.tile import TileContext

    nc = bacc.Bacc(None, target_bir_lowering=False, debug=True)

    inT = nc.declare_dram_parameter("inT", [F, W], mybir.dt.float16, isOutput=False)
    Tm = nc.declare_dram_parameter("Tm", [F, KD], mybir.dt.float16, isOutput=False)
    # dmat[5m+d, m] = 2.0 (d-sum of 2*relu), dmat[5m+d, 32+m] = 1.0 (S row sums)
    dmat = nc.declare_dram_parameter(
        "dmat", [CHUNK, 64], mybir.dt.float16, isOutput=False
    )
    negI = nc.declare_dram_parameter("negI", [128, 128], mybir.dt.float16, isOutput=False)
    rowsum = nc.declare_dram_parameter("rowsum", [128, JPC], mybir.dt.float32, isOutput=True)
    colout = nc.declare_dram_parameter(
        "colout", [128, NCOL], mybir.dt.float32, isOutput=True
    )

    with TileContext(nc) as tc:
        with tc.tile_pool(name="persist", bufs=1) as pp:
            T_sb = pp.tile([128, 8 * KD], mybir.dt.float16, name="T_sb")
            inT_sb = pp.tile([128, 8 * W], mybir.dt.float16, name="inT_sb")
            dmat_sb = pp.tile([CHUNK, 64], mybir.dt.float16, name="dmat_sb")
            negI_sb = pp.tile([128, 128], mybir.dt.float16, name="negI_sb")
            S16_sb = pp.tile([128, W], mybir.dt.float16, name="S16_sb")
            negSj_sb = pp.tile([128, JPC], mybir.dt.float32, name="negSj_sb")
            colacc_sb = pp.tile([128, NCOL], mybir.dt.float32, name="colacc_sb")
            xT_sb = pp.tile([128, NCHUNK * W], mybir.dt.float16, name="xT_sb")
            # f32 upcasts of xT columns 0..JPC (tensor_scalar per-partition
            # scalars must be f32), from the fp16 xT so the diagonal is 0
            xTj_sb = pp.tile([128, NCHUNK * JPC], mybir.dt.float32, name="xTj_sb")
            raw_sb = pp.tile([128, JPC], mybir.dt.float32, name="raw_sb")
            warm_sb = pp.tile([1, 1], mybir.dt.float32, name="warm_sb")

            # Static rings: every (j, chunk) gets its own ab tile; the dump
            # ring must cover the tail rows reduced on DVE after the loop.
            ab_ring = [
                pp.tile(
                    [CHUNK, _erow(t // NCHUNK) - (t // NCHUNK)],
                    mybir.dt.float16,
                    name=f"ab{t}",
                )
                for t in range(JPC * NCHUNK)
            ]
            NDUMP = 24
            dump_ring = [
                pp.tile([128, W], mybir.dt.float16, name=f"dump{t}")
                for t in range(NDUMP)
            ]
            # discard targets for the tail-row sum tensor_scalars (two, so
            # consecutive ones don't chain a same-tile WAW into a 2nd wait)
            scr_sb = [
                pp.tile([128, W], mybir.dt.float16, name=f"scr{t}") for t in range(2)
            ]

            # --- input DMAs, spread across the three DMA-capable queues ---
            def dma_T(e, h):
                e.dma_start(
                    out=T_sb[:, h * 4 * KD : (h + 1) * 4 * KD].rearrange(
                        "p (t k) -> p t k", t=4
                    ),
                    in_=Tm[h * 512 : (h + 1) * 512, :].rearrange(
                        "(t p) k -> p t k", t=4
                    ),
                )

            def dma_inT(e, h):
                e.dma_start(
                    out=inT_sb[:, h * 4 * W : (h + 1) * 4 * W].rearrange(
                        "p (t w) -> p t w", t=4
                    ),
                    in_=inT[h * 512 : (h + 1) * 512, :].rearrange(
                        "(t p) w -> p t w", t=4
                    ),
                )

            dma_T(nc.sync, 0)
            dma_inT(nc.gpsimd, 0)
            dma_T(nc.gpsimd, 1)
            dma_inT(nc.sync, 1)
            nc.sync.dma_start(out=dmat_sb[:, :], in_=dmat[:, :])
            nc.sync.dma_start(out=negI_sb[:, :], in_=negI[:, :])
            # zero the Pool-side column-sum accumulator while DMAs run
            nc.gpsimd.memset(colacc_sb[:, :], 0.0)
            # warm the ACT exp table while the xT matmuls run (~1.3us)
            nc.vector.memset(warm_sb[:, :], 0.0)
            nc.scalar.activation(
                warm_sb[:, :], warm_sb[:, :], mybir.ActivationFunctionType.Exp
            )

            with tc.tile_pool(name="xtps", bufs=3, space="PSUM") as xtps:
                # single-wait gates: absorb each input-DMA semaphore so no
                # real matmul carries two waits (a 2-wait matmul gets an
                # EventSemaphore that resets the PE p-state ramp, halving
                # the input-stage matmul clock)
                gate_srcs = [
                    T_sb[:, 0:64],
                    inT_sb[:, 0:64],
                    T_sb[:, 4 * KD : 4 * KD + 64],
                    inT_sb[:, 4 * W : 4 * W + 64],
                    dmat_sb[:, 0:64],
                    negI_sb[:, 0:64],
                ]
                g_ps0 = xtps.tile([128, 64], mybir.dt.float32, name="gate_a", bufs=1)
                g_ps1 = xtps.tile([128, 64], mybir.dt.float32, name="gate_b", bufs=1)
                for gi, gsrc in enumerate(gate_srcs):
                    g_ps = g_ps0 if gi < 4 else g_ps1
                    q = gi % 4
                    nc.tensor.matmul(
                        g_ps[32 * q : 32 * q + 32, :],
                        gsrc[:, 0:32],
                        gsrc[:, 0:64],
                        start=True,
                        stop=True,
                        tile_position=(0, 32 * q),
                        skip_group_check=True,
                    )

                # --- xT chunks: xT[kd, i] via PE over f tiles; each chunk's
                # S-row-sum quadrant matmul is emitted right after its copy
                # so the S16/negSj chain never serializes at the end ---
                S_ps = xtps.tile([128, W], mybir.dt.float32, name="S_ps", bufs=1)
                for c in range(NCHUNK):
                    xt_ps = xtps.tile([CHUNK, W], mybir.dt.float32, name="xt_ps")
                    for t in range(8):
                        nc.tensor.matmul(
                            xt_ps[:, :],
                            T_sb[:, t * KD + c * CHUNK : t * KD + (c + 1) * CHUNK],
                            inT_sb[:, t * W : (t + 1) * W],
                            start=(t == 0),
                            stop=(t == 7),
                        )
                    # all fp16 copies on DVE: ACT is the binding engine and
                    # DVE's late-stream reduce stalls absorb the extra work
                    nc.vector.tensor_copy(
                        xT_sb[0:CHUNK, c * W : (c + 1) * W], xt_ps[:, :]
                    )
                    nc.vector.tensor_copy(
                        xTj_sb[0:CHUNK, c * JPC : (c + 1) * JPC],
                        xT_sb[0:CHUNK, c * W : c * W + JPC],
                    )
                    # S[k, i] = sum_d x[i,k,d] at partitions 32c+m
                    nc.tensor.matmul(
                        S_ps[32 * c : 32 * c + 32, :],
                        dmat_sb[:, 32:64],
                        xT_sb[0:CHUNK, c * W : (c + 1) * W],
                        start=True,
                        stop=True,
                        tile_position=(0, 32 * c),
                        skip_group_check=True,
                    )
                    # per-quadrant S16 copy so the fp16 S is complete right
                    # after the last chunk's S matmul; split across ACT and
                    # DVE to take 0.9us off the binding ACT queue
                    if c % 2 == 0:
                        nc.scalar.copy(
                            S16_sb[32 * c : 32 * c + 32, :],
                            S_ps[32 * c : 32 * c + 32, :],
                        )
                    else:
                        nc.vector.tensor_copy(
                            S16_sb[32 * c : 32 * c + 32, :],
                            S_ps[32 * c : 32 * c + 32, :],
                        )
                # exp bias column: -S_j, upcast from the SAME fp16 S16 the
                # negI matmul reads so the diagonal cancels exactly; on ACT
                # so the DVE queue is never stalled ahead of the first ts
                nc.scalar.mul(negSj_sb[:, :], S16_sb[:, 0:JPC], -1.0)

            mainps_es = contextlib.ExitStack()
            mainps = mainps_es.enter_context(
                tc.tile_pool(name="mainps", bufs=1, space="PSUM")
            )
            NDIST = 6
            dist_bufs = [
                mainps.tile([128, W], mybir.dt.float32, name=f"dist{i}")
                for i in range(NDIST)
            ]

            def emit_ts(j, c, eng):
                E = _erow(j)
                eng.tensor_scalar(
                    ab_ring[j * NCHUNK + c][:, :],
                    xT_sb[0:CHUNK, c * W + j : c * W + E],
                    xTj_sb[0:CHUNK, c * JPC + j : c * JPC + j + 1],
                    0.0,
                    mybir.AluOpType.subtract,
                    mybir.AluOpType.max,
                )

            # chunk-3 relu for EVEN rows runs on the otherwise-idle Pool
            # engine (~440ns there vs ~125ns of DVE time saved; the total
            # is DVE-work-bound). Emitted 4 rows ahead so the PE d-sum
            # never waits on the slower Pool.
            emit_ts(0, NCHUNK - 1, nc.gpsimd)
            emit_ts(2, NCHUNK - 1, nc.gpsimd)

            # --- main loop over output rows ---
            for j in range(JPC):
                E = _erow(j)
                # colacc: diag (lower-tri by symmetry), k123, and the
                # distance-4 cols STRICTLY above the antidiagonal (the col
                # range ends one short of E so the t=j partner, already in
                # the own row sum, is not double-counted via the exchange)
                CP = W - 1 - j
                dist = dist_bufs[j % NDIST]
                if j + 4 < JPC and (j + 4) % 2 == 0:
                    emit_ts(j + 4, NCHUNK - 1, nc.gpsimd)
                for c in range(NCHUNK):
                    # ab = relu(xT[:, i] - xT[:, j]) : (in - s1) max 0.0
                    # (const scalar2 keeps the second DVE read port free so
                    # the 4x perf mode applies)
                    if not (c == NCHUNK - 1 and j % 2 == 0):
                        emit_ts(j, c, nc.vector)
                    # dist[32c+m, :] = 2 * sum_d ab[5m+d, :]
                    nc.tensor.matmul(
                        dist[32 * c : 32 * c + 32, j:E],
                        dmat_sb[:, 0:32],
                        ab_ring[j * NCHUNK + c][:, :],
                        start=True,
                        stop=False,
                        tile_position=(0, 32 * c),
                        skip_group_check=True,
                    )
                # dist += -S[k, i], LAST so early rows don't stall on the
                # S16 chain during the input stage
                nc.tensor.matmul(
                    dist[:, j:E],
                    negI_sb[:, :],
                    S16_sb[:, j:E],
                    start=False,
                    stop=True,
                    skip_group_check=True,
                )
                # dump = exp(-dist - S_j) = exp(-L1(i,j)) fp16; ACT accum
                # gives the row sums except for the last NRED rows (DVE
                # tensor_reduces after the loop save the fixed 187ns ACT
                # accumulator-read there)
                nc.scalar.activation(
                    dump_ring[j % NDUMP][:, j:E],
                    dist[:, j:E],
                    mybir.ActivationFunctionType.Exp,
                    bias=negSj_sb[:, j : j + 1],
                    scale=-1.0,
                    accum_out=(raw_sb[:, j : j + 1] if j < JPC - NRED else None),
                )
                if j == 33:
                    # these outputs are final: row sums 0..32 (ACT accum),
                    # colacc diag cols 0..32 (adds stop at j=i) and cols
                    # 288..320 (dist-4 cols for partners <32, adds stop at
                    # j=31) — ship them now to shrink the end-of-loop DMA
                    nc.sync.dma_start(out=rowsum[:, 0:32], in_=raw_sb[:, 0:32])
                    nc.sync.dma_start(out=colout[:, 0:32], in_=colacc_sb[:, 0:32])
                    nc.sync.dma_start(out=colout[:, D4LO:W], in_=colacc_sb[:, D4LO:W])
                # column sums on the otherwise-idle Pool (GPSIMD) engine:
                # colacc += dump[:, j:CP] (f32 accumulator in SBUF)
                nc.gpsimd.tensor_tensor(
                    colacc_sb[:, j:CP],
                    colacc_sb[:, j:CP],
                    dump_ring[j % NDUMP][:, j:CP],
                    mybir.AluOpType.add,
                )
            for j in range(JPC - NRED, JPC):
                # row sum via tensor_scalar accum_out: (dump + 0) reduced
                # with op1=add — keeps the DVE 4x perf mode (tensor_reduce
                # runs at 1x, ~2.7x slower)
                nc.vector.tensor_scalar(
                    scr_sb[j % 2][:, j : _erow(j)],
                    dump_ring[j % NDUMP][:, j : _erow(j)],
                    0.0,
                    None,
                    mybir.AluOpType.add,
                    mybir.AluOpType.add,
                    accum_out=raw_sb[:, j : j + 1],
                )
            mainps_es.close()
            nc.sync.dma_start(out=rowsum[:, 32:JPC], in_=raw_sb[:, 32:JPC])
            nc.gpsimd.dma_start(out=colout[:, 32:D4LO], in_=colacc_sb[:, 32:D4LO])

    nc.finalize()
    return nc


def _aux_consts():
    dm = np.zeros([CHUNK, 64], dtype=np.float16)
    for m in range(KPC):
        dm[5 * m : 5 * m + 5, m] = 2.0
        dm[5 * m : 5 * m + 5, 32 + m] = 1.0
    negI = (-np.eye(128)).astype(np.float16)
    return dm, negI


def make_in_maps(inputs, T):
    f16 = np.float16
    Tm = np.asarray(T, dtype=np.float32).astype(f16)
    dm, negI = _aux_consts()
    in_maps = []
    x = np.asarray(inputs, dtype=np.float32)
    for c in range(NCORES):
        rolled = np.roll(x, -JPC * c, axis=0)[0:W, :]
        # local col order: [diag+k123 (0..256) | c+4 rows 63..0 descending
        # (256..320): col 256+s holds c+4's row 63-s]
        rolled = np.concatenate([rolled[0:D4HI], rolled[D4HI:W][::-1]], axis=0)
        inTc = np.ascontiguousarray(rolled.T).astype(f16)
        in_maps.append(
            {
                "inT": inTc,
                "Tm": Tm,
                "dmat": dm,
                "negI": negI,
            }
        )
    return in_maps


def assemble_output(results):
    out = np.zeros([B, K], dtype=np.float32)
    # own row sums: raw[32c+m, j] -> out[64q+j, 25c+m]
    for q in range(NCORES):
        raw = np.asarray(results[q]["rowsum"], dtype=np.float32)  # [128, JPC]
        for cc in range(NCHUNK):
            out[JPC * q : JPC * (q + 1), KPC * cc : KPC * (cc + 1)] = raw[
                32 * cc : 32 * cc + KPC, :
            ].T
    # column sums: core b's group k serves rows of core b+k. k=0 is the own
    # diag block (lower triangle by symmetry; subtract the double-counted
    # self term exp(0)=1), k=1..3 are the exchanged off-diag groups.
    for b in range(NCORES):
        col = np.asarray(results[b]["colout"], dtype=np.float32)  # [128, NCOL]
        for k in range(0, NEX + 1):
            q = (b + k) % NCORES
            blk = col[:, JPC * k : JPC * (k + 1)]  # [128, JPC]
            for cc in range(NCHUNK):
                out[JPC * q : JPC * (q + 1), KPC * cc : KPC * (cc + 1)] += blk[
                    32 * cc : 32 * cc + KPC, :
                ].T
        # distance-4 block (descending): col 256+s serves row 63-s of core
        # b+4 with colsums over core b's rows j < 63-s (strict antidiagonal)
        q = (b + 4) % NCORES
        blk = col[:, D4HI:W][:, ::-1]  # reversed: col t serves row t of b+4
        for cc in range(NCHUNK):
            out[JPC * q : JPC * (q + 1), KPC * cc : KPC * (cc + 1)] += blk[
                32 * cc : 32 * cc + KPC, :
            ].T
    out -= 1.0
    return out


def kernel(inputs, T):
    from concourse.bass_utils import run_bass_kernel_spmd

    if "nc" not in _NC_CACHE:
        _NC_CACHE["nc"] = build_nc()
    nc = _NC_CACHE["nc"]
    in_maps = make_in_maps(inputs, T)
    res = run_bass_kernel_spmd(nc, in_maps, list(range(NCORES)))
    return assemble_output(res.results)


if __name__ == "__main__":
    sys.path.insert(0, "/root/problem")
    from reference import setup_inputs, reference

    inputs = setup_inputs()
    expected = np.asarray(reference(**inputs))
    actual = kernel(**{k: np.asarray(v) for k, v in inputs.items()})
    err = np.abs(actual - expected)
    rel = np.linalg.norm(actual - expected) / np.linalg.norm(expected)
    print(f"max abs err: {err.max():.3e}")
    print(f"Relative error: {rel:.3e}")
